# revision 10
# baseline (speedup 1.0000x reference)
"""BlockMoE Trainium2 kernel (8 NeuronCores, pure data parallel).

Reference computation (per row b of x [B=65536, 1024]):
  gate:    g = relu(x @ gw1 + gb1); w = softmax(g @ gw2 + gb2)   [B, 64]
  experts: xb = x.reshape(B, 64, 16)
           h1 = relu(xb[:,e] @ ew1[e] + eb1[e])                  [B, 64, 64]
           h2 = relu(h1 @ ew2[e] + eb2[e])                       [B, 64, 32]
           l  = h2 @ ew3[e] + eb3[e]                             [B, 64]
  out:     sum_e w[:,e] * l[:,e]                                 [B, 1]

Strategy:
  - Shard batch across 8 cores (8192 rows each), replicate params.
  - Host pre-transposes x per shard into [16 tiles, 8 chunks, 128, 512]
    blocks so the kernel streams x^T directly (no PE transposes).
  - All matmuls run in float32r (full 1 col/cycle PE rate, ~1e-4 rel
    rounding vs fp32's 4 cycles/col).
  - Batch lives in the matmul free dim (N=512); contraction dims live on
    partitions, so layer outputs chain into the next layer without any
    transposes.  Per-expert weights are packed block-diagonally on host:
      L1: 32 lhsT of [128, 128] (2 experts each, K=features, M=2x64 hid)
      L2: 16 groups x 2 accumulating lhsT of [128, 128] (4 experts/group)
      L3: 16 accumulating lhsT of [128, 64] into one [64, 512] logits PSUM
  - softmax-combine folded into exp / sums:
      out = (ones^T (eg * l)) / (ones^T eg),  eg = exp(gate logits)
"""

import sys

sys.path.insert(0, "/opt/trn_rl_repo")

import numpy as np

import concourse.bass as bass
import concourse.mybir as mybir
import concourse.tile as tile
from concourse.bass_utils import run_bass_kernel_spmd

NCORES = 8
B = 65536
FULL = 1024
E = 64
WBLK = 16  # expert input block width
HID = 64
GH = 32
BL = B // NCORES  # rows per core
RT = 512  # rows per tile
NT = BL // RT  # tiles per core (16)
NCH = FULL // 128  # x^T chunks per tile (8)

F32 = mybir.dt.float32
F32R = mybir.dt.bfloat16
import ml_dtypes
BF = ml_dtypes.bfloat16
AF = mybir.ActivationFunctionType
ALU = mybir.AluOpType


def _split_multi_waits(nc, max_waits=1):
    # This walrus build rejects >1 sync-wait on one instruction; move the
    # excess onto fresh EventSemaphore instructions placed just before.
    ctr = 0
    for f in nc.m.functions:
        for blk in f.blocks:
            new_list, changed = [], False
            for inst in blk.instructions:
                si = inst.sync_info
                if si is not None and si.on_wait and len(si.on_wait) > max_waits:
                    waits = list(si.on_wait)
                    excess, keep = waits[:-max_waits], waits[-max_waits:]
                    for w in excess:
                        ev = mybir.InstEventSemaphore(
                            name=f"splitw_{ctr}", ins=[], outs=[]
                        )
                        ctr += 1
                        ev.engine = inst.engine
                        ev.sync_info = mybir.SyncInfo(on_wait=[w], on_update=[])
                        new_list.append(ev)
                    si.on_wait = keep
                    changed = True
                new_list.append(inst)
            if changed:
                blk.instructions = new_list


def _pack_params(gw1, gb1, gw2, gb2, ew1, eb1, ew2, eb2, ew3, eb3):
    """Pack parameters into the SBUF layouts the kernel DMAs verbatim."""
    # gate layer 1: lhsT chunks [128, 32] laid out as [128, 8*32]
    G1 = np.zeros((NCH, 128, 128), np.float32)
    G1[:, :, 0:GH] = gw1.reshape(NCH, 128, GH)
    gw1s = np.ascontiguousarray(G1.transpose(1, 0, 2).reshape(128, NCH * 128))
    G2 = np.zeros((128, 128), np.float32)
    G2[0:GH, 0:E] = gw2
    OC = np.zeros((128, 128), np.float32)
    OC[:, 0] = 1.0
    # L1: pair i = 4c + j covers experts (8c+2j, 8c+2j+1); rhs = x^T chunk c.
    # lhsT is K=128 zero-padded outside rows [32j, 32j+32).
    W1 = np.zeros((32, 128, 128), np.float32)
    for i in range(32):
        c, j = divmod(i, 4)
        e0 = 8 * c + 2 * j
        W1[i, 32 * j : 32 * j + 16, 0:64] = ew1[e0]
        W1[i, 32 * j + 16 : 32 * j + 32, 64:128] = ew1[e0 + 1]
    W1s = np.ascontiguousarray(W1.transpose(1, 0, 2).reshape(128, 32 * 128))
    # L2: chunk k = 2q + c; rhs = h1 tile (2q+c) holding experts
    # (4q+2c, 4q+2c+1); out partitions 32e..32e+32 = expert 4q+e.
    W2 = np.zeros((32, 128, 128), np.float32)
    for q in range(16):
        for c in range(2):
            k = 2 * q + c
            W2[k, 0:64, 64 * c : 64 * c + 32] = ew2[4 * q + 2 * c]
            W2[k, 64:128, 64 * c + 32 : 64 * c + 64] = ew2[4 * q + 2 * c + 1]
    W2s = np.ascontiguousarray(W2.transpose(1, 0, 2).reshape(128, 32 * 128))
    # L3: chunk q; rhs = h2 tile q (experts 4q..4q+3, 32 partitions each);
    # lhsT col (4q+e) = ew3[4q+e].
    W3 = np.zeros((16, 128, 128), np.float32)
    for q in range(16):
        for e in range(4):
            W3[q, 32 * e : 32 * e + 32, 4 * q + e] = ew3[4 * q + e][:, 0]
    W3s = np.ascontiguousarray(W3.transpose(1, 0, 2).reshape(128, 16 * 128))
    # biases as per-partition columns
    eb1s = np.ascontiguousarray(eb1.reshape(32, 128).T)  # [128, 32]
    eb2s = np.ascontiguousarray(eb2.reshape(16, 128).T)  # [128, 16]
    eb3s = np.ascontiguousarray(eb3.reshape(64, 1))  # [64, 1]
    gb1s = np.ascontiguousarray(gb1.reshape(GH, 1))  # [32, 1]
    gb2s = np.ascontiguousarray(gb2.reshape(E, 1))  # [64, 1]
    ones = np.ones((E, 1), np.float32)
    return {
        "gw1": gw1s.astype(BF),
        "gw2": np.ascontiguousarray(G2).astype(BF),  # [128, 128] padded
        "onescol": OC.astype(BF),
        "w1": W1s.astype(BF),
        "w2": W2s.astype(BF),
        "w3": W3s.astype(BF),
        "eb1": eb1s,
        "eb2": eb2s,
        "eb3": eb3s,
        "gb1": gb1s,
        "gb2": gb2s,
        "ones": ones.astype(BF),
    }


def _build_nc(split=True):
    nc = bass.Bass()
    xt = nc.declare_dram_parameter("xt", [NT, NCH, 128, RT], F32R, isOutput=False)
    w1 = nc.declare_dram_parameter("w1", [128, 32 * 128], F32R, isOutput=False)
    w2 = nc.declare_dram_parameter("w2", [128, 32 * 128], F32R, isOutput=False)
    w3 = nc.declare_dram_parameter("w3", [128, 16 * 128], F32R, isOutput=False)
    gw1 = nc.declare_dram_parameter("gw1", [128, NCH * 128], F32R, isOutput=False)
    gw2 = nc.declare_dram_parameter("gw2", [128, 128], F32R, isOutput=False)
    onescol = nc.declare_dram_parameter("onescol", [128, 128], F32R, isOutput=False)
    ones = nc.declare_dram_parameter("ones", [E, 1], F32R, isOutput=False)
    eb1 = nc.declare_dram_parameter("eb1", [128, 32], F32, isOutput=False)
    eb2 = nc.declare_dram_parameter("eb2", [128, 16], F32, isOutput=False)
    eb3 = nc.declare_dram_parameter("eb3", [E, 1], F32, isOutput=False)
    gb1 = nc.declare_dram_parameter("gb1", [GH, 1], F32, isOutput=False)
    gb2 = nc.declare_dram_parameter("gb2", [E, 1], F32, isOutput=False)
    # y[t, 0:RT] = numerator, y[t, RT:2RT] = denominator; host divides.
    y = nc.declare_dram_parameter("y", [NT, 2 * RT], F32, isOutput=True)

    with tile.TileContext(nc) as tc:
        with (
            tc.tile_pool(name="consts", bufs=1) as consts,
            tc.tile_pool(name="xp", bufs=32) as xpool,
            tc.tile_pool(name="h1s", bufs=8) as h1pool,
            tc.tile_pool(name="h2s", bufs=5) as h2pool,
            tc.tile_pool(name="gsb", bufs=3) as gpool,
            tc.tile_pool(name="ph1", bufs=3, space="PSUM") as ph1,
            tc.tile_pool(name="ph2", bufs=2, space="PSUM") as ph2,
            tc.tile_pool(name="plg", bufs=2, space="PSUM") as plg,
            tc.tile_pool(name="pgate", bufs=1, space="PSUM") as pgate,
        ):
            # ---- load constants (big expert weights first: L1 of tile 0
            # needs w1 earliest after the gate phase)
            w1t = consts.tile([128, 32, 128], F32R)
            nc.sync.dma_start(w1t[:], w1[:].rearrange("p (i m) -> p i m", i=32))
            xts0 = []
            for _c in range(NCH):
                x0c = xpool.tile([128, RT], F32R, tag="xt", name="x0c")
                nc.sync.dma_start(x0c[:], xt[0, _c])
                xts0.append(x0c)
            w2t = consts.tile([128, 32, 128], F32R)
            nc.sync.dma_start(w2t[:], w2[:].rearrange("p (i m) -> p i m", i=32))
            w3t = consts.tile([128, 16, 128], F32R)
            nc.sync.dma_start(w3t[:], w3[:].rearrange("p (i m) -> p i m", i=16))
            gw1t = consts.tile([128, NCH, 128], F32R)
            nc.sync.dma_start(gw1t[:], gw1[:].rearrange("p (c m) -> p c m", c=NCH))
            gw2t = consts.tile([128, 128], F32R)
            nc.sync.dma_start(gw2t[:], gw2[:])
            onect = consts.tile([128, 128], F32R)
            nc.sync.dma_start(onect[:], onescol[:])
            g1sp = [
                consts.tile([128, RT], F32R, tag=f"g1sp{k}", name=f"g1sp{k}")
                for k in range(2)
            ]
            egp = [
                consts.tile([128, RT], F32R, tag=f"egp{k}", name=f"egp{k}")
                for k in range(2)
            ]
            mp = [
                consts.tile([128, RT], F32R, tag=f"mp{k}", name=f"mp{k}")
                for k in range(2)
            ]
            for tl in (*g1sp, *egp, *mp):
                nc.gpsimd.memzero(tl[:])
            onest = consts.tile([E, 1], F32R)
            nc.sync.dma_start(onest[:], ones[:])
            eb1t = consts.tile([128, 32], F32)
            nc.sync.dma_start(eb1t[:], eb1[:])
            eb2t = consts.tile([128, 16], F32)
            nc.sync.dma_start(eb2t[:], eb2[:])
            eb3t = consts.tile([E, 1], F32)
            nc.sync.dma_start(eb3t[:], eb3[:])
            gb1t = consts.tile([GH, 1], F32)
            nc.sync.dma_start(gb1t[:], gb1[:])
            gb2t = consts.tile([E, 1], F32)
            nc.sync.dma_start(gb2t[:], gb2[:])

            def relu_bias(out_t, psum_t, bias_ap, use_act):
                if use_act:
                    nc.scalar.activation(out_t[:], psum_t[:], AF.Relu, bias=bias_ap)
                else:
                    nc.vector.tensor_scalar(
                        out_t[:], psum_t[:], bias_ap, 0.0, ALU.add, ALU.max
                    )

            def issue_x(t):
                tiles = []
                for c in range(NCH):
                    xc = xpool.tile([128, RT], F32R, tag="xt")
                    nc.sync.dma_start(xc[:], xt[t, c])
                    tiles.append(xc)
                return tiles

            # PE warmup: ~4us of dummy matmuls so the HAM clock gate opens
            # while the first x tiles and weights are still streaming in.
            dummy = xpool.tile([128, RT], F32R, tag="xt")
            nc.gpsimd.memzero(dummy[:])
            wp = ph1.tile([128, RT], F32, tag="h1p")
            for _ in range(24):
                nc.tensor.matmul(
                    wp[:], dummy[:, 0:128], dummy[:], start=True, stop=True
                )

            xts = xts0
            pending = None
            for t in range(NT):
                xts_next = issue_x(t + 1) if t + 1 < NT else None

                # ---- gate layer 1.  For t==0 the whole gate is deferred
                # until after the expert stream: gate1 needs gw1 + ALL x
                # chunks (ready ~21us in) while L1 needs only w1 + chunk 0
                # (~10us) -- deferring starts the PE ~8us earlier.
                def emit_gate1():
                    g1p = pgate.tile([128, RT], F32, tag="pg", name="g1p")
                    for cc in range(NCH):
                        nc.tensor.matmul(
                            g1p[:],
                            gw1t[:, cc, :],
                            xts[cc][:],
                            start=(cc == 0),
                            stop=(cc == NCH - 1),
                        )
                    g1s = g1sp[t % 2]
                    nc.scalar.activation(
                        g1s[0:GH, :], g1p[0:GH, :], AF.Relu, bias=gb1t[:, 0:1]
                    )
                    return g1s

                def emit_gate2(g1s):
                    g2p = ph2.tile([128, RT], F32, tag="h2p", name="g2p")
                    nc.tensor.matmul(
                        g2p[:], gw2t[:], g1s[:], start=True, stop=True
                    )
                    eg = egp[t % 2]
                    nc.scalar.activation(
                        eg[0:E, :], g2p[0:E, :], AF.Exp, bias=gb2t[:, 0:1]
                    )
                    return eg

                def emit_den(eg):
                    denp = ph2.tile([128, RT], F32, tag="h2p", name="denp")
                    nc.tensor.matmul(
                        denp[:], onect[:], eg[:], start=True, stop=True
                    )
                    o = gpool.tile([1, 2 * RT], F32, tag="o", name="o")
                    nc.vector.tensor_copy(o[:, RT : 2 * RT], denp[0:1, :])
                    return o

                if t > 0:
                    g1s = emit_gate1()

                # ---- experts
                lgp = plg.tile([128, RT], F32, tag="lg")
                eg = None
                for c in range(NCH):
                    if c == 1 and t > 0:
                        eg = emit_gate2(g1s)
                    if c == 2 and t > 0:
                        o = emit_den(eg)
                    if c == 3 and pending is not None:
                        # deferred combine tail of the previous tile: by now
                        # ls/m have long finished, so the PE never stalls
                        tp, mp_, op = pending
                        nump = ph2.tile([128, RT], F32, tag="h2p")
                        nc.tensor.matmul(
                            nump[:], onect[:], mp_[:], start=True, stop=True
                        )
                        nc.scalar.copy(op[:, 0:RT], nump[0:1, :])
                        nc.sync.dma_start(y[tp : tp + 1, :], op[:])
                        pending = None
                    for d in range(2):  # duo of L1 pairs -> one L2 group
                        q = 2 * c + d
                        h1s_duo = []
                        for j2 in range(2):
                            j = 2 * d + j2
                            i = 4 * c + j
                            h1p = ph1.tile([128, RT], F32, tag="h1p")
                            nc.tensor.matmul(
                                h1p[:],
                                w1t[:, i, :],
                                xts[c][:],
                                start=True,
                                stop=True,
                            )
                            h1s = h1pool.tile([128, RT], F32R, tag="h1s")
                            relu_bias(h1s, h1p, eb1t[:, i : i + 1], use_act=(j % 2 == 0))
                            h1s_duo.append(h1s)
                        h2p = ph2.tile([128, RT], F32, tag="h2p")
                        nc.tensor.matmul(
                            h2p[:],
                            w2t[:, 2 * q, :],
                            h1s_duo[0][:],
                            start=True,
                            stop=False,
                        )
                        nc.tensor.matmul(
                            h2p[:],
                            w2t[:, 2 * q + 1, :],
                            h1s_duo[1][:],
                            start=False,
                            stop=True,
                        )
                        h2s = h2pool.tile([128, RT], F32R, tag="h2s")
                        relu_bias(h2s, h2p, eb2t[:, q : q + 1], use_act=(q % 2 == 1))
                        nc.tensor.matmul(
                            lgp[:],
                            w3t[:, q, :],
                            h2s[:],
                            start=(q == 0),
                            stop=(q == 15),
                        )

                if t == 0:
                    g1s = emit_gate1()
                    eg = emit_gate2(g1s)
                    o = emit_den(eg)

                # ---- combine head; the num matmul + output DMA are
                # deferred into the next tile's warm PE stream
                ls = gpool.tile([E, RT], F32R, tag="ls")
                nc.scalar.activation(ls[:], lgp[0:E, :], AF.Identity, bias=eb3t[:, 0:1])
                m = mp[t % 2]
                nc.gpsimd.tensor_mul(m[0:E, :], eg[0:E, :], ls[:])
                pending = (t, m, o)
                xts = xts_next
            tp, mp_, op = pending
            nump = ph2.tile([128, RT], F32, tag="h2p")
            nc.tensor.matmul(nump[:], onect[:], mp_[:], start=True, stop=True)
            nc.scalar.copy(op[:, 0:RT], nump[0:1, :])
            nc.sync.dma_start(y[tp : tp + 1, :], op[:])

    if split:
        _split_multi_waits(nc)
    return nc


def _shard_x(x):
    """Per-core blocked transpose: [BL, 1024] -> [NT, NCH, 128, RT]."""
    shards = []
    for s in range(NCORES):
        xs = x[s * BL : (s + 1) * BL]  # [8192, 1024]
        blk = xs.reshape(NT, RT, NCH, 128).transpose(0, 2, 3, 1)
        shards.append(np.ascontiguousarray(blk).astype(BF))
    return shards


def run(inputs, trace=False):
    x = np.asarray(inputs["x"], np.float32)
    params = _pack_params(
        np.asarray(inputs["gw1"], np.float32),
        np.asarray(inputs["gb1"], np.float32),
        np.asarray(inputs["gw2"], np.float32),
        np.asarray(inputs["gb2"], np.float32),
        np.asarray(inputs["ew1"], np.float32),
        np.asarray(inputs["eb1"], np.float32),
        np.asarray(inputs["ew2"], np.float32),
        np.asarray(inputs["eb2"], np.float32),
        np.asarray(inputs["ew3"], np.float32),
        np.asarray(inputs["eb3"], np.float32),
    )
    xshards = _shard_x(x)
    nc = _build_nc()
    in_maps = [{"xt": xshards[s], **params} for s in range(NCORES)]
    res = run_bass_kernel_spmd(nc, in_maps, list(range(NCORES)), trace=trace)
    outs = []
    for s in range(NCORES):
        ys = res.results[s]["y"].reshape(NT, 2, RT)  # numerator, denominator
        outs.append((ys[:, 0, :] / ys[:, 1, :]).reshape(BL, 1))
    return np.concatenate(outs, axis=0), res


def kernel(**inputs) -> np.ndarray:
    out, _ = run(inputs, trace=False)
    return out

